# revision 12
# baseline (speedup 1.0000x reference)
"""Trainium2 Bass kernel for a 7-layer ternary-weight (BitNet) 1D conv
feature extractor with exact-erf GELU after each layer.

Contract: kernel(**inputs) takes the FULL inputs from setup_inputs()
(x: [8, 160000] f32, w0..w6 / b0..b6 conv params) and returns the full
output [8, 256, 500] f32.

Strategy: data-parallel over batch - one batch element per NeuronCore.
Weights are ternarized on host (sign in {-1,0,1} exact in fp16; the
absmean scale folds into the GELU's per-partition scale). Activations
are fp16 in SBUF; convs are accumulating matmuls into fp32 PSUM,
GELU on the ACT engine at full 128 lanes.

Key layout decision: every intermediate activation is stored
PHASE-SPLIT - even positions (e) and odd positions (o) in separate
tiles. A stride-2 conv consuming phase-split input reads each tap as a
CONTIGUOUS range (chunk0 units) or stride-2 (phase-split-output units)
- never stride-4, which streams at half rate through the PE. The ACT
engine performs the e/o scatter for free via strided PSUM-source reads
(two calls per chunk0 unit); L0's input phase buffer is pre-split on
the host so L0 needs no scatter.

PE-array tiling:
 - L0 (contraction 10): 2 concurrent row tiles (tile_position (0,0) /
   (64,0)) per 1024-col supertile.
 - cout=192 layers (L1-L3): couts 128-191 are produced phase-split
   (psum parts 0-63 = even outs, 64-127 = odd) by col-tiled matmul
   pairs (0,0)+(0,64) that run concurrently (stride-2 rhs reads).
 - cin=192 layers (L2-L4): the leftover cin 128-191 (stored as
   X=[e;o] + shifted D=[o_-1;o] + dup E=[.;e]) is consumed as a comb
   matmul (taps 0+2 stacked, contraction 128) plus an e-tap
   (contraction 64) row-tile-paired across two output slices.
"""

import numpy as np

# (in_ch, out_ch, kernel, stride, pad) - fixed problem geometry
LAYERS = [(1, 128, 10, 5, 4), (128, 192, 3, 2, 1), (192, 192, 3, 2, 1),
          (192, 192, 3, 2, 1), (192, 256, 3, 2, 1), (256, 256, 4, 2, 1),
          (256, 256, 4, 2, 1)]
T_IN = 160000
LOUT = [32000, 16000, 8000, 4000, 2000, 1000, 500]
N_CORES = 8
NT = 512
A0C = 8192      # L0-output ring chunk (position space)
A0H = A0C // 2  # ... in phase columns
NGRP = {0: 1, 1: 3, 2: 5, 3: 5, 4: 5, 5: 8, 6: 8}


def _wlayout():
    wcols = {}
    tot = 0
    for i, (cin, cout, k, s, p) in enumerate(LAYERS):
        wcols[i] = tot
        tot += NGRP[i] * cout if i else 128
    return wcols, tot


def _pack_host(ws, bs):
    """Ternarize weights; pack signs (fp16) and bias+scale (fp32)."""
    wcols, tot = _wlayout()
    wpk = np.zeros((128, tot), np.float16)
    bpk = np.zeros((128, 26), np.float32)
    bcol = 0
    for i, (cin, cout, k, s, p) in enumerate(LAYERS):
        w = np.asarray(ws[i], np.float32)
        scale = max(float(np.mean(np.abs(w))), 1e-5)
        sign = np.clip(np.round(w / scale), -1.0, 1.0)  # [cout, cin, k]
        base = wcols[i]
        if i == 0:
            blk = sign[:, 0, :].T.astype(np.float16)  # [10, 128]
            wpk[0:k, base:base + 128] = blk
            wpk[64:64 + k, base:base + 128] = blk
        elif cin == 128:
            for kk in range(3):
                wpk[0:128, base + kk * cout:base + (kk + 1) * cout] = \
                    sign[:, :, kk].T.astype(np.float16)
        elif cin == 192:
            for kk in range(3):
                wpk[0:128, base + kk * cout:base + (kk + 1) * cout] = \
                    sign[:, 0:128, kk].T.astype(np.float16)
            c3 = base + 3 * cout   # comb: [tap0 over o[t-1]; tap2 over o[t]]
            wpk[0:64, c3:c3 + cout] = sign[:, 128:192, 0].T
            wpk[64:128, c3:c3 + cout] = sign[:, 128:192, 2].T
            c4 = base + 4 * cout   # etap: tap1 over e[t], duplicated rows
            wpk[0:64, c4:c4 + cout] = sign[:, 128:192, 1].T
            wpk[64:128, c4:c4 + cout] = sign[:, 128:192, 1].T
        else:  # cin 256
            g = 0
            for ti in range(2):
                for kk in range(k):
                    wpk[0:128, base + g * cout:base + (g + 1) * cout] = \
                        sign[:, 128 * ti:128 * ti + 128, kk].T
                    g += 1
        b = np.asarray(bs[i], np.float32)
        bpk[0:128, bcol] = b[0:128]
        bpk[0:128, bcol + 1] = scale
        bcol += 2
        if cout > 128:
            if cout == 192:   # stacked for phase-split psum
                bpk[0:64, bcol] = b[128:192]
                bpk[64:128, bcol] = b[128:192]
            else:
                bpk[0:128, bcol] = b[128:256]
            bpk[0:128, bcol + 1] = scale
            bcol += 2
    return wpk, bpk


def _bcol(i, mi):
    """Column of (bias, scale) pair for layer i, cout-chunk mi."""
    c = 0
    for j in range(i):
        c += 2 if LAYERS[j][1] <= 128 else 4
    return c + 2 * mi


def _prep_x(xb):
    """Per-core L0 input, phase-reordered: cols [0:16000) hold the
    window for even L0 outputs, [16000:) for odd. xr[j, col] =
    xpad[5t + j] with t = 2*col (col<16000) or 2*(col-16000)+1."""
    xpad = np.zeros(T_IN + 16, np.float16)
    xpad[4:4 + T_IN] = xb.astype(np.float16)
    L = LOUT[0]
    xr = np.empty((10, L), np.float16)
    for j in range(10):
        xr[j, :] = xpad[j:j + 5 * L:5]
    return np.concatenate([xr[:, 0::2], xr[:, 1::2]], axis=1)


_CACHE = {}


def _build():
    """Build + compile the Bass program (weight-data-independent)."""
    if "nc" in _CACHE:
        return _CACHE["nc"]
    from concourse import bacc
    import concourse.mybir as mybir
    import concourse.tile as tile

    F16 = mybir.dt.float16
    F32 = mybir.dt.float32
    GELU = mybir.ActivationFunctionType.Gelu
    wcols, tot = _wlayout()

    nc = bacc.Bacc("TRN2")
    xr_d = nc.dram_tensor("xr", [10, LOUT[0]], F16, kind="ExternalInput")
    wp_d = nc.dram_tensor("wp", [128, tot], F16, kind="ExternalInput")
    bp_d = nc.dram_tensor("bp", [128, 26], F32, kind="ExternalInput")
    y_d = nc.dram_tensor("y", [256, 500], F32, kind="ExternalOutput")

    with tile.TileContext(nc) as tc:
        pools = []

        def mkpool(name, bufs=1, space="SBUF"):
            p = tc.alloc_tile_pool(name=name, bufs=bufs, space=space)
            pools.append(p)
            return p

        wpool = mkpool("wpool")
        wt = wpool.tile([128, tot], F16, name="wt")
        bt = wpool.tile([128, 26], F32, name="bt")

        opool = mkpool("opool")
        stage = opool.tile([128, 1000], F32, name="stage")
        scratch = opool.tile([128, 512], F16, name="scratch")
        xpool = mkpool("xpool", bufs=3)
        a0pool = mkpool("a0pool", bufs=2)

        # Phase-split activation stores. Ne/No tiles: col 0 = halo
        # (phase position -1; zero for o, unused for e), position t at
        # col t+1. X/D/E: col t = phase col t (no offset).
        lpool = mkpool("lpool")
        N1e = lpool.tile([128, 8004], F16, name="N1e")
        N1o = lpool.tile([128, 8004], F16, name="N1o")
        X1 = lpool.tile([128, 8002], F16, name="X1")
        D1 = lpool.tile([128, 8002], F16, name="D1")
        E1 = lpool.tile([128, 8002], F16, name="E1")
        N2e = lpool.tile([128, 4004], F16, name="N2e")
        N2o = lpool.tile([128, 4004], F16, name="N2o")
        X2 = lpool.tile([128, 4002], F16, name="X2")
        D2 = lpool.tile([128, 4002], F16, name="D2")
        E2 = lpool.tile([128, 4002], F16, name="E2")
        N3e = lpool.tile([128, 2004], F16, name="N3e")
        N3o = lpool.tile([128, 2004], F16, name="N3o")
        X3 = lpool.tile([128, 2002], F16, name="X3")
        D3 = lpool.tile([128, 2002], F16, name="D3")
        N4ae = lpool.tile([128, 1004], F16, name="N4ae")
        N4ao = lpool.tile([128, 1004], F16, name="N4ao")
        N4be = lpool.tile([128, 1004], F16, name="N4be")
        N4bo = lpool.tile([128, 1004], F16, name="N4bo")
        N5ae = lpool.tile([128, 504], F16, name="N5ae")
        N5ao = lpool.tile([128, 504], F16, name="N5ao")
        N5be = lpool.tile([128, 504], F16, name="N5be")
        N5bo = lpool.tile([128, 504], F16, name="N5bo")

        for t in (N1o, N2o, N3o, N4ao, N4bo, N5ao, N5bo):
            nc.vector.memset(t[:, 0:1], 0.0)
        for t in (N4ae, N4be):          # L5 e-tap kk3 reads e[1000]
            nc.vector.memset(t[:, 1001:1002], 0.0)
        for t in (N5ae, N5be):          # L6 e-tap kk3 reads e[500]
            nc.vector.memset(t[:, 501:502], 0.0)
        for t in (D1, D2, D3):
            nc.vector.memset(t[0:64, 0:1], 0.0)

        # PSUM phase A: L0 [128,1024] (2 banks, 2-way row tiling) +
        # "fa" [128,2048] chunk0 units + "fb" [128,1024] chunk1 units.
        poolL0 = tc.alloc_tile_pool(name="poolL0", bufs=1, space="PSUM")
        poolFA = tc.alloc_tile_pool(name="poolFA", bufs=1, space="PSUM")
        poolFB = tc.alloc_tile_pool(name="poolFB", bufs=1, space="PSUM")
        cur = {"fa": poolFA, "fb": poolFB, "faw": 2048, "fbw": 1024}

        def fa_tile():
            return cur["fa"].tile([128, cur["faw"]], F32, name="pfa",
                                  tag="fa")

        def fb_tile():
            return cur["fb"].tile([128, cur["fbw"]], F32, name="pfb",
                                  tag="fb")

        # PE warm-up junk matmuls (HAM clock-gate + first-DMA window)
        nc.vector.memset(scratch[:, :], 0.0)

        def junk_mms(n):
            jp = fb_tile()
            for _ in range(n):
                nc.tensor.matmul(jp[:, 0:512], scratch[:, 0:128],
                                 scratch[:, :], start=True, stop=True)

        junk_mms(14)

        def act(dst, ps, i, mi):
            c = _bcol(i, mi)
            nc.scalar.activation(dst, ps, GELU,
                                 bias=bt[0:128, c:c + 1],
                                 scale=bt[0:128, c + 1:c + 2])

        def act_eo(dste, dsto, ps, n, i, mi):
            """Phase-scatter GELU: two strided-psum-source calls."""
            act(dste, ps[0:128, 0:n:2], i, mi)
            act(dsto, ps[0:128, 1:n:2], i, mi)

        # ---------------- L1 units (inputs: a0 ring chunk) -------------
        a0e_t = [None] * 4
        a0o_t = [None] * 4
        l1b = wcols[1]

        def l1_c0_unit(c, u0, n):
            """L1 couts 0-127 for outputs [u0, u0+n), n <= 2048 even.
            Taps (all contiguous reads): kk0=o[u-1], kk1=e[u], kk2=o[u]."""
            cb = c * A0H
            srcs = [(a0o_t[c], u0 - 1), (a0e_t[c], u0), (a0o_t[c], u0)]
            ps = fa_tile()
            for kk in range(3):
                lhsT = wt[0:128, l1b + 192 * kk:l1b + 192 * kk + 128]
                src, p = srcs[kk]
                for s0 in range(0, n, NT):
                    w = min(NT, n - s0)
                    col = p + s0 - cb + 1
                    nc.tensor.matmul(ps[:, s0:s0 + w], lhsT,
                                     src[0:128, col:col + w],
                                     start=(kk == 0), stop=(kk == 2))
            act_eo(N1e[0:128, 1 + u0 // 2:1 + (u0 + n) // 2],
                   N1o[0:128, 1 + u0 // 2:1 + (u0 + n) // 2],
                   ps, n, 1, 0)

        def l1_c1_unit(c, v0, n):
            """L1 couts 128-191 phase-split for outputs [v0, v0+n):
            col-tiled pairs, stride-2 reads from the a0 phase tiles."""
            cb = c * A0H
            h = n // 2
            p0 = v0 // 2
            ps = fb_tile()
            # (src, base pos) for mmE / mmO per kk
            se = [(a0o_t[c], v0 - 1), (a0e_t[c], v0), (a0o_t[c], v0)]
            so = [(a0o_t[c], v0), (a0e_t[c], v0 + 1), (a0o_t[c], v0 + 1)]
            for kk in range(3):
                lhsT = wt[0:128, l1b + 192 * kk + 128:l1b + 192 * kk + 192]
                for b0 in range(0, h, NT):
                    hw = min(NT, h - b0)
                    sE, pE = se[kk]
                    sO, pO = so[kk]
                    cE = pE + 2 * b0 - cb + 1
                    cO = pO + 2 * b0 - cb + 1
                    nc.tensor.matmul(ps[0:64, b0:b0 + hw], lhsT,
                                     sE[0:128, cE:cE + 2 * hw:2],
                                     start=(kk == 0), stop=(kk == 2),
                                     tile_position=(0, 0))
                    nc.tensor.matmul(ps[64:128, b0:b0 + hw], lhsT,
                                     sO[0:128, cO:cO + 2 * hw:2],
                                     start=(kk == 0), stop=(kk == 2),
                                     tile_position=(0, 64))
            act(X1[0:128, p0:p0 + h], ps[0:128, 0:h], 1, 1)
            nc.sync.dma_start(out=D1[64:128, p0:p0 + h],
                              in_=X1[64:128, p0:p0 + h])
            lo = max(p0, 1)
            nc.sync.dma_start(out=D1[0:64, lo:p0 + h],
                              in_=X1[64:128, lo - 1:p0 + h - 1])
            nc.sync.dma_start(out=E1[64:128, p0:p0 + h],
                              in_=X1[0:64, p0:p0 + h])

        # ------------- deep units (L2-L4, comb/etap scheme) ------------
        DIN = {2: (N1e, N1o, X1, D1, E1), 3: (N2e, N2o, X2, D2, E2),
               4: (N3e, N3o, X3, D3, None)}
        DOUT = {2: (N2e, N2o, X2, D2, E2), 3: (N3e, N3o, X3, D3, None)}

        def deep_c0_unit(i, u0, n, mi):
            """Layer i in 2..4, couts [128mi,+128), outs [u0,u0+n).
            All rhs reads contiguous."""
            Ne, No, Xi, Di, Ei = DIN[i]
            base = wcols[i]
            cout = LAYERS[i][1]
            m0 = 128 * mi
            srcs = [(No, u0 - 1), (Ne, u0), (No, u0)]
            ps = fa_tile()
            for kk in range(3):
                lhsT = wt[0:128, base + kk * cout + m0:
                          base + kk * cout + m0 + 128]
                src, p = srcs[kk]
                for s0 in range(0, n, NT):
                    w = min(NT, n - s0)
                    col = p + s0 + 1
                    nc.tensor.matmul(ps[:, s0:s0 + w], lhsT,
                                     src[0:128, col:col + w],
                                     start=(kk == 0), stop=False)
            lhsT = wt[0:128, base + 3 * cout + m0:base + 3 * cout + m0 + 128]
            for s0 in range(0, n, NT):
                w = min(NT, n - s0)
                nc.tensor.matmul(ps[:, s0:s0 + w], lhsT,
                                 Di[0:128, u0 + s0:u0 + s0 + w],
                                 start=False, stop=False)
            ecol = base + 4 * cout + m0
            for si, s0 in enumerate(range(0, n, NT)):
                w = min(NT, n - s0)
                if Ei is not None and si % 2 == 1:
                    nc.tensor.matmul(ps[:, s0:s0 + w],
                                     wt[64:128, ecol:ecol + 128],
                                     Ei[64:128, u0 + s0:u0 + s0 + w],
                                     start=False, stop=True)
                else:
                    nc.tensor.matmul(ps[:, s0:s0 + w],
                                     wt[0:64, ecol:ecol + 128],
                                     Xi[0:64, u0 + s0:u0 + s0 + w],
                                     start=False, stop=True)
            if i < 4:
                de, do = DOUT[i][0], DOUT[i][1]
            elif mi == 0:
                de, do = N4ae, N4ao
            else:
                de, do = N4be, N4bo
            act_eo(de[0:128, 1 + u0 // 2:1 + (u0 + n) // 2],
                   do[0:128, 1 + u0 // 2:1 + (u0 + n) // 2],
                   ps, n, i, mi)

        def deep_c1_unit(i, v0, n):
            """Layer i in 2..3, couts 128-191 phase-split, outs
            [v0, v0+n): col-tiled pairs, stride-2 reads."""
            Ne, No, Xi, Di, Ei = DIN[i]
            _, _, Xo, Do, Eo = DOUT[i]
            base = wcols[i]
            cout = LAYERS[i][1]
            h = n // 2
            p0 = v0 // 2
            ps = fb_tile()

            def pair(lhsT, rhsE, rhsO, b0, hw, start, stop):
                nc.tensor.matmul(ps[0:64, b0:b0 + hw], lhsT, rhsE,
                                 start=start, stop=stop,
                                 tile_position=(0, 0))
                nc.tensor.matmul(ps[64:128, b0:b0 + hw], lhsT, rhsO,
                                 start=start, stop=stop,
                                 tile_position=(0, 64))

            se = [(No, v0 - 1), (Ne, v0), (No, v0)]
            so = [(No, v0), (Ne, v0 + 1), (No, v0 + 1)]
            for kk in range(3):
                lhsT = wt[0:128, base + kk * cout + 128:base + kk * cout + 192]
                for b0 in range(0, h, NT):
                    hw = min(NT, h - b0)
                    sE, pE = se[kk]
                    sO, pO = so[kk]
                    cE = pE + 2 * b0 + 1
                    cO = pO + 2 * b0 + 1
                    pair(lhsT, sE[0:128, cE:cE + 2 * hw:2],
                         sO[0:128, cO:cO + 2 * hw:2],
                         b0, hw, kk == 0, False)
            lhsT = wt[0:128, base + 3 * cout + 128:base + 3 * cout + 192]
            for b0 in range(0, h, NT):
                hw = min(NT, h - b0)
                vb = v0 + 2 * b0
                pair(lhsT, Di[0:128, vb:vb + 2 * hw:2],
                     Di[0:128, vb + 1:vb + 1 + 2 * hw:2],
                     b0, hw, False, False)
            lhsT = wt[0:64, base + 4 * cout + 128:base + 4 * cout + 192]
            for b0 in range(0, h, NT):
                hw = min(NT, h - b0)
                vb = v0 + 2 * b0
                pair(lhsT, Xi[0:64, vb:vb + 2 * hw:2],
                     Xi[0:64, vb + 1:vb + 1 + 2 * hw:2],
                     b0, hw, False, True)
            act(Xo[0:128, p0:p0 + h], ps[0:128, 0:h], i, 1)
            nc.sync.dma_start(out=Do[64:128, p0:p0 + h],
                              in_=Xo[64:128, p0:p0 + h])
            lo = max(p0, 1)
            nc.sync.dma_start(out=Do[0:64, lo:p0 + h],
                              in_=Xo[64:128, lo - 1:p0 + h - 1])
            if Eo is not None:
                nc.sync.dma_start(out=Eo[64:128, p0:p0 + h],
                                  in_=Xo[0:64, p0:p0 + h])

        # ---------------- L5/L6 std units ------------------------------
        SRC56 = {5: ((N4ae, N4ao), (N4be, N4bo)),
                 6: ((N5ae, N5ao), (N5be, N5bo))}

        def std_unit(i, mi):
            """k=4 layer: taps o[v-1], e[v], o[v], e[v+1], contiguous."""
            (ae, ao), (be, bo) = SRC56[i]
            base = wcols[i]
            lout = LOUT[i]
            ps = fa_tile() if i == 5 else fb_tile()
            g = 0
            for ti in range(2):
                e, o = (ae, ao) if ti == 0 else (be, bo)
                srcs = [(o, -1), (e, 0), (o, 0), (e, 1)]
                for kk in range(4):
                    lhsT = wt[0:128, base + g * 256 + 128 * mi:
                              base + g * 256 + 128 * mi + 128]
                    src, p = srcs[kk]
                    for s0 in range(0, lout, NT):
                        w = min(NT, lout - s0)
                        col = p + s0 + 1
                        nc.tensor.matmul(ps[:, s0:s0 + w], lhsT,
                                         src[0:128, col:col + w],
                                         start=(g == 0), stop=(g == 7))
                    g += 1
            if i == 5:
                de, do = (N5ae, N5ao) if mi == 0 else (N5be, N5bo)
                act_eo(de[0:128, 1:1 + lout // 2],
                       do[0:128, 1:1 + lout // 2], ps, lout, i, mi)
            else:
                act(stage[0:128, 500 * mi:500 * mi + lout],
                    ps[0:128, 0:lout], i, mi)

        # ============ phase A: L0 streamed; L1 + early-L2 woven ========
        wrest = [0]

        def after_first_xt():
            if wrest[0] == 1:
                mid = wcols[4]
                nc.gpsimd.dma_start(out=wt[:, 128:mid],
                                    in_=wp_d.ap()[:, 128:mid])
                nc.gpsimd.dma_start(out=wt[:, mid:tot],
                                    in_=wp_d.ap()[:, mid:tot])
            wrest[0] += 1

        n_ch = (LOUT[0] + A0C - 1) // A0C
        for c in range(n_ch):
            cb2 = c * A0H
            csz2 = min(A0H, LOUT[0] // 2 - cb2)
            ate = a0pool.tile([128, A0H + 5], F16, tag="a0e", name=f"ae{c}")
            ato = a0pool.tile([128, A0H + 5], F16, tag="a0o", name=f"ao{c}")
            a0e_t[c], a0o_t[c] = ate, ato
            if c == 0:
                nc.vector.memset(ato[:, 0:1], 0.0)
                nc.vector.memset(ate[:, 0:1], 0.0)
            else:
                nc.vector.tensor_copy(ato[:, 0:1],
                                      a0o_t[c - 1][:, A0H:A0H + 1])
                nc.vector.tensor_copy(ate[:, 0:1],
                                      a0e_t[c - 1][:, A0H:A0H + 1])
            # fillers: L1 of chunk c-1, then early L2 once inputs exist
            fillers = []
            if c > 0:
                pb = (c - 1) * A0C // 2
                for j in range(2):
                    u0 = pb + 2048 * j
                    nn = min(2048, LOUT[1] - u0)
                    fillers.append(lambda c=c - 1, u0=u0, nn=nn:
                                   l1_c0_unit(c, u0, nn))
                    fillers.append(lambda c=c - 1, u0=u0, nn=nn:
                                   l1_c1_unit(c, u0, nn))
            if c >= 2:
                u0 = (c - 2) * 2048
                fillers.append(lambda u0=u0: deep_c0_unit(2, u0, 2048, 0))
                fillers.append(lambda u0=u0: deep_c1_unit(2, u0, 2048))
            if c == 0:
                fillers = [lambda: junk_mms(2) for _ in range(4)]
            fi = 0
            nst = 2 * ((csz2 + 1023) // 1024)
            for sti in range(nst):
                ph = sti % 2
                t0 = cb2 + 1024 * (sti // 2)
                stw = min(1024, cb2 + csz2 - t0)
                dst = (ate, ato)[ph]
                xt = xpool.tile([128, NT], F16, tag="xt",
                                name=f"xt{ph}_{t0}")
                if t0 == 0 and ph == 0:
                    nc.sync.dma_start(out=wt[:, 0:128],
                                      in_=wp_d.ap()[:, 0:128])
                    nc.sync.dma_start(out=bt[:, :], in_=bp_d.ap())
                for s in range(0, stw, NT):
                    w = min(NT, stw - s)
                    g = (s // NT) * 64
                    nc.sync.dma_start(
                        out=xt[g:g + 10, 0:w],
                        in_=xr_d.ap()[:, 16000 * ph + t0 + s:
                                      16000 * ph + t0 + s + w])
                after_first_xt()
                ps = poolL0.tile([128, 1024], F32, name="pl0", tag="l0")
                for s in range(0, stw, NT):
                    w = min(NT, stw - s)
                    g = (s // NT) * 64
                    nc.tensor.matmul(ps[:, s:s + w],
                                     wt[g:g + 10, 0:128],
                                     xt[g:g + 10, 0:w],
                                     start=True, stop=True,
                                     tile_position=(g, 0))
                act(dst[0:128, t0 - cb2 + 1:t0 - cb2 + 1 + stw],
                    ps[0:128, 0:stw], 0, 0)
                if sti % 2 == 1 and fi < len(fillers):
                    fillers[fi]()
                    fi += 1
            while fi < len(fillers):
                fillers[fi]()
                fi += 1
        # drain: L1 of last chunk, then L2/L3/... in phase B
        pb = (n_ch - 1) * A0C // 2
        for j in range(2):
            u0 = pb + 2048 * j
            if u0 >= LOUT[1]:
                break
            nn = min(2048, LOUT[1] - u0)
            l1_c0_unit(n_ch - 1, u0, nn)
            l1_c1_unit(n_ch - 1, u0, nn)

        # ============ phase B: rest of L2, then L3..L6 =================
        poolFB.release()
        poolFA.release()
        poolL0.release()
        poolC0 = tc.alloc_tile_pool(name="poolC0", bufs=2, space="PSUM")
        poolC1 = tc.alloc_tile_pool(name="poolC1", bufs=1, space="PSUM")
        cur["fa"] = poolC0
        cur["fb"] = poolC1
        cur["faw"] = 1024
        cur["fbw"] = 2048

        deep_c0_unit(2, 4096, 1024, 0)
        deep_c1_unit(2, 4096, 3904)
        deep_c0_unit(2, 5120, 1024, 0)
        deep_c0_unit(2, 6144, 1024, 0)
        deep_c0_unit(2, 7168, 832, 0)
        deep_c0_unit(3, 0, 1024, 0)
        deep_c1_unit(3, 0, 4000)
        deep_c0_unit(3, 1024, 1024, 0)
        deep_c0_unit(3, 2048, 1024, 0)
        deep_c0_unit(3, 3072, 928, 0)
        deep_c0_unit(4, 0, 1024, 0)
        deep_c0_unit(4, 0, 1024, 1)
        deep_c0_unit(4, 1024, 976, 0)
        deep_c0_unit(4, 1024, 976, 1)
        std_unit(5, 0)
        std_unit(5, 1)
        std_unit(6, 0)
        std_unit(6, 1)

        nc.sync.dma_start(out=y_d.ap()[0:128, :], in_=stage[:, 0:500])
        nc.sync.dma_start(out=y_d.ap()[128:256, :], in_=stage[:, 500:1000])
        poolC1.release()
        poolC0.release()
        for p in reversed(pools):
            p.release()

    nc.compile()
    _CACHE["nc"] = nc
    return nc


def kernel(x, w0, b0, w1, b1, w2, b2, w3, b3, w4, b4, w5, b5, w6, b6):
    import os
    from concourse.bass_utils import run_bass_kernel_spmd

    ws = [w0, w1, w2, w3, w4, w5, w6]
    bs = [b0, b1, b2, b3, b4, b5, b6]
    wpk, bpk = _pack_host(ws, bs)
    x = np.asarray(x, np.float32)
    in_maps = [{"xr": _prep_x(x[b]), "wp": wpk, "bp": bpk}
               for b in range(N_CORES)]
    nc = _build()
    trace = bool(os.environ.get("BITCONV_TRACE"))
    res = run_bass_kernel_spmd(nc, in_maps, core_ids=list(range(N_CORES)),
                               trace=trace)
    if trace:
        print(f"HW exec time: {res.exec_time_ns} ns")
        _CACHE["last_results"] = res
    return np.stack([res.results[b]["y"] for b in range(N_CORES)], axis=0)
